# revision 9
# baseline (speedup 1.0000x reference)
"""Trainium2 Bass kernel for nn_CrossAttentionFusion (dense_transformer).

Pure data parallel over 8 NeuronCores (batch 32768 -> 4096/core), 32 tiles of
128 rows each.  Row-major residual stream in bf16; attention on the Vector
engine with packed-bf16 access patterns (2x/4x DVE modes); matmuls on PE in
bf16 (activation-stationary for QKV/Wo/W2, weight-stationary for W1 so the
gelu output is directly the W2 lhsT).  LN1's per-row rstd is folded into the
softmax (rstd_i*rstd_j on scores, rstd_j into prob) so LN1's apply never
materializes.  All Scalar-engine activations draw from one table set
(ln+exp): rsqrt = exp(-0.5*ln(v+eps)); gelu is an erf-polynomial on DVE.
Residual adds and small copies ride the otherwise-idle GpSimd engine.
"""

import contextlib
import ctypes
import math
import os
import sys
import types
from contextlib import ExitStack

import numpy as np
import ml_dtypes

import concourse.bass as bass
import concourse.tile as tile
from concourse import mybir
from concourse.bass_utils import run_bass_kernel_spmd
from concourse.masks import make_identity


def _install_ntff_hook_shim():
    """Provide antenv.axon_hooks if the image lacks it, so trace=True works."""
    try:
        import antenv.axon_hooks  # noqa: F401
        return
    except ImportError:
        pass
    so_path = "/opt/axon/libaxon_pjrt.so"
    hook = None
    if os.path.exists(so_path):
        try:
            lib = ctypes.CDLL(so_path)
            if hasattr(lib, "axon_start_nrt_profile"):
                lib.axon_start_nrt_profile.argtypes = [
                    ctypes.POINTER(ctypes.c_int64), ctypes.c_size_t]
                lib.axon_start_nrt_profile.restype = ctypes.c_int64
                lib.axon_stop_nrt_profile.argtypes = [ctypes.c_char_p]
                lib.axon_stop_nrt_profile.restype = ctypes.c_int64

                @contextlib.contextmanager
                def _hook(output_dir, device_ids):
                    import jax
                    jax.devices()
                    if device_ids:
                        ids = (ctypes.c_int64 * len(device_ids))(*device_ids)
                        rc = lib.axon_start_nrt_profile(ids, len(device_ids))
                    else:
                        rc = lib.axon_start_nrt_profile(None, 0)
                    if rc != 0:
                        raise RuntimeError(f"axon_start_nrt_profile rc={rc}")
                    try:
                        yield
                    finally:
                        n = lib.axon_stop_nrt_profile(str(output_dir).encode())
                        print(f"ntff profile: {n} file(s) -> {output_dir}",
                              file=sys.stderr)

                hook = _hook
        except OSError:
            pass

    mod = types.ModuleType("antenv.axon_hooks")
    mod.get_axon_ntff_profile_hook = lambda: hook
    mod.set_axon_ntff_profile_hook = lambda h: None
    sys.modules["antenv.axon_hooks"] = mod


_install_ntff_hook_shim()

# Problem shapes (hardcoded per contract).
D, H, HD, FF, L, SYM, B = 256, 8, 32, 256, 3, 64, 32768
NCORES = 8
BC = B // NCORES          # 4096 rows per core
P = 128                   # SBUF partitions
NT = BC // P              # 32 tiles per core
F32 = mybir.dt.float32
BF16 = mybir.dt.bfloat16
AF = mybir.ActivationFunctionType
OP = mybir.AluOpType
AX = mybir.AxisListType
EPS = 1e-5
SCALE = 1.0 / math.sqrt(HD)
GA = 1.702  # unused (erf-poly gelu); kept for reference

# odd-polynomial fit of erf(z/sqrt(2)) on |z|<=2.6 (max err 3e-3; the gelu
# input z1 has std ~0.32 so 6-sigma is ~1.9)
ERF_A1 = 0.79397813
ERF_A3 = -0.12376735
ERF_A5 = 0.013831441
ERF_A7 = -6.7821721e-4

BF = ml_dtypes.bfloat16


def _ln_rstd(nc, work, mv_var_ap, n, eps_ap, tag, bias_ap=0.0):
    """rstd = exp(-0.5*ln(var+eps) + bias) on Scalar (single-table)."""
    lnv = work.tile([P, n], F32, tag=tag + "_lnv")
    nc.scalar.activation(out=lnv, in_=mv_var_ap, func=AF.Ln,
                         bias=eps_ap, scale=1.0)
    rstd = work.tile([P, n], F32, tag=tag + "_rstd")
    nc.scalar.activation(out=rstd, in_=lnv, func=AF.Exp, scale=-0.5,
                         bias=bias_ap)
    return rstd


def _stats4(nc, work, x, tag):
    """bn stats for 4 groups of 256. Returns mv [P,4,2] (mean,var)."""
    st = work.tile([P, 4, 6], F32, tag=tag + "_st")
    for g in range(4):
        nc.vector.bn_stats(out=st[:, g, :], in_=x[:, g, :])
    mv = work.tile([P, 4, 2], F32, tag=tag + "_mv")
    for g in range(4):
        nc.vector.bn_aggr(out=mv[:, g, :], in_=st[:, g, :])
    return mv


def build_kernel(nc):
    # Per-core data inputs (host pre-adds token-type emb, casts to bf16,
    # zero-pads sym_feat 64->128 and x slot 2).
    xin = nc.dram_tensor("xin", [BC, 4, D], BF16, kind="ExternalInput").ap()
    sfp = nc.dram_tensor("sfp", [BC, P], BF16, kind="ExternalInput").ap()
    # Replicated weights, bf16, pre-chunked for 128-partition contractions.
    symw = nc.dram_tensor("symw", [P, D], BF16, kind="ExternalInput").ap()
    wqkv = nc.dram_tensor("wqkv", [L, 2, P, 3 * D], BF16, kind="ExternalInput").ap()
    wo = nc.dram_tensor("wo", [L, 2, P, D], BF16, kind="ExternalInput").ap()
    w1 = nc.dram_tensor("w1", [L, 2, 2, P, P], BF16, kind="ExternalInput").ap()
    w2 = nc.dram_tensor("w2", [L, 2, P, D], BF16, kind="ExternalInput").ap()
    vecb = nc.dram_tensor("vecb", [1, D], BF16, kind="ExternalInput").ap()  # symbt
    out = nc.dram_tensor("out", [BC, D], F32, kind="ExternalOutput").ap()

    with ExitStack() as ctx:
        tc = ctx.enter_context(tile.TileContext(nc))
        singles = ctx.enter_context(tc.tile_pool(name="singles", bufs=1))
        work = ctx.enter_context(tc.tile_pool(name="work", bufs=4))
        xpool = ctx.enter_context(tc.tile_pool(name="xpool", bufs=2))
        xcpool = ctx.enter_context(tc.tile_pool(name="xcpool", bufs=2))
        lhstp = ctx.enter_context(tc.tile_pool(name="lhst", bufs=2))
        qkpool = ctx.enter_context(tc.tile_pool(name="qkpool", bufs=2))
        vtpool = ctx.enter_context(tc.tile_pool(name="vtpool", bufs=2))
        attw = ctx.enter_context(tc.tile_pool(name="attw", bufs=2))
        opool = ctx.enter_context(tc.tile_pool(name="opool", bufs=2))
        glpool = ctx.enter_context(tc.tile_pool(name="glpool", bufs=2))
        tpsum = ctx.enter_context(tc.tile_pool(name="tpsum", bufs=2, space="PSUM"))
        mmpsum = ctx.enter_context(tc.tile_pool(name="mmpsum", bufs=3, space="PSUM"))

        # ---- constants / resident weights ----
        identb = singles.tile([P, P], BF16)
        make_identity(nc, identb)
        eps_t = singles.tile([P, 1], F32)
        nc.vector.memset(eps_t, EPS)
        zero_t = singles.tile([P, 1], F32)
        nc.vector.memset(zero_t, 0.0)
        lnq_t = singles.tile([P, 1], F32)
        nc.vector.memset(lnq_t, math.log(0.25))
        symw_sb = singles.tile([P, D], BF16)
        nc.gpsimd.dma_start(out=symw_sb, in_=symw)
        wqkv_sb = singles.tile([P, L, 2, 3 * D], BF16)
        nc.gpsimd.dma_start(out=wqkv_sb, in_=wqkv.transpose([2, 0, 1, 3]))
        wo_sb = singles.tile([P, L, 2, D], BF16)
        nc.gpsimd.dma_start(out=wo_sb, in_=wo.transpose([2, 0, 1, 3]))
        w1_sb = singles.tile([P, L, 2, 2, P], BF16)
        nc.gpsimd.dma_start(out=w1_sb, in_=w1.transpose([3, 0, 1, 2, 4]))
        w2_sb = singles.tile([P, L, 2, D], BF16)
        nc.gpsimd.dma_start(out=w2_sb, in_=w2.transpose([2, 0, 1, 3]))
        symbt_sb = singles.tile([P, 1, D], BF16)
        nc.sync.dma_start(out=symbt_sb, in_=vecb.partition_broadcast(P))

        def transpose8(src, dst, tag, copy_engine):
            """src: [P, 4(i), 2(c), 128] bf16 view; dst: [P, 2(c), 4(i), 128]
            SBUF tile with dst[:, c, i, :] = src[:, i, c, :].T"""
            for c in range(2):
                pt = tpsum.tile([P, 4, P], BF16, tag="tp")
                for i in range(4):
                    nc.tensor.transpose(pt[:, i, :], src[:, i, c, :], identb)
                ce = getattr(nc, copy_engine)
                if copy_engine == "scalar":
                    ce.copy(out=dst[:, c], in_=pt)
                else:
                    with nc.allow_low_precision(reason="bf16 lhsT copy"):
                        ce.tensor_copy(out=dst[:, c], in_=pt)

        for it in range(NT):
            row = it * P
            # ---- build x [P,4,D] bf16 (slot 2 incoming zeros; sym fills it) --
            x = xpool.tile([P, 4, D], BF16, tag="x")
            nc.sync.dma_start(out=x, in_=xin[row:row + P])
            sft = work.tile([P, P], BF16, tag="sft")
            nc.sync.dma_start(out=sft, in_=sfp[row:row + P])

            # sym branch: x2 = LN(sf @ symW) + symbt   (sym_ln_g==1 asserted host)
            pt = tpsum.tile([P, 4, P], BF16, tag="tp")
            nc.tensor.transpose(pt[:, 0, :], sft, identb)
            sfT = work.tile([P, P], BF16, tag="sfT")
            nc.scalar.copy(out=sfT, in_=pt[:, 0, :])
            mm = mmpsum.tile([P, 2, 512], F32, tag="mm")
            zsym = mm[:, 0, 0:D]
            nc.tensor.matmul(zsym, sfT, symw_sb, start=True, stop=True)
            st6 = work.tile([P, 6], F32, tag="sym_st")
            nc.vector.bn_stats(out=st6, in_=zsym)
            mvs = work.tile([P, 2], F32, tag="sym_mv")
            nc.vector.bn_aggr(out=mvs, in_=st6)
            rstd_s = _ln_rstd(nc, work, mvs[:, 1:2], 1, eps_t[:, :1], "sym", zero_t[:, :1])
            zn = work.tile([P, D], BF16, tag="sym_zn")
            nc.vector.tensor_scalar(out=zn, in0=zsym, scalar1=mvs[:, 0:1],
                                    scalar2=rstd_s[:, 0:1], op0=OP.subtract,
                                    op1=OP.mult)
            with nc.allow_low_precision(reason="bf16 residual stream"):
                nc.vector.tensor_tensor(x[:, 2, :], zn, symbt_sb[:, 0, :], OP.add)

            # ---- transformer layers ----
            for l in range(L):
                # LN1 stats; apply is folded into attention scalars.
                mv1 = _stats4(nc, work, x, f"ln1_{l}")
                rstd1 = _ln_rstd(nc, work, mv1[:, :, 1], 4, eps_t[:, :1], f"r1_{l}", zero_t[:, :1])
                xc = xcpool.tile([P, 4, D], BF16, tag="xc")
                with nc.allow_low_precision(reason="centered acts bf16"):
                    for g in range(4):
                        nc.gpsimd.tensor_scalar(
                            out=xc[:, g, :], in0=x[:, g, :],
                            scalar1=mv1[:, g, 0:1], scalar2=None,
                            op0=OP.subtract)
                # xcT [P, 2(c), 4(i), 128]
                xcT = lhstp.tile([P, 2, 4, P], BF16, tag="lhst")
                transpose8(xc.rearrange("p i (c f) -> p i c f", c=2), xcT,
                           "xcT", "scalar")

                # qkv per token i: q|k -> qk sbuf, v -> vt[h,d,j=i]
                qk = qkpool.tile([P, 4, 512], BF16, tag="qk")
                vt = vtpool.tile([P, H, HD, 4], BF16, tag="vt")
                for i in range(4):
                    mmi = mmpsum.tile([P, 2, 512], F32, tag="mm")
                    for c in range(2):
                        nc.tensor.matmul(mmi[:, 0, :], xcT[:, c, i, :],
                                         wqkv_sb[:, l, c, 0:512],
                                         start=(c == 0), stop=(c == 1))
                    for c in range(2):
                        nc.tensor.matmul(mmi[:, 1, 0:D], xcT[:, c, i, :],
                                         wqkv_sb[:, l, c, 512:768],
                                         start=(c == 0), stop=(c == 1))
                    nc.scalar.copy(out=qk[:, i, :], in_=mmi[:, 0, :])
                    nc.scalar.copy(
                        out=vt[:, :, :, i],
                        in_=mmi[:, 1, 0:D].rearrange("p (h d) -> p h d", h=H))

                # ---- attention (row-major, packed bf16) ----
                q = qk[:, :, 0:D]       # [P, i, (h d)]
                k = qk[:, :, D:2 * D]   # [P, j, (h d)]
                prod = attw.tile([P, 4, 4, D], BF16, tag="att_prod")  # [i,j,hd]
                qb = q[:, :, None, :].to_broadcast((P, 4, 4, D))
                kb = k[:, None, :, :].to_broadcast((P, 4, 4, D))
                with nc.allow_low_precision(reason="attn bf16"):
                    nc.vector.tensor_tensor(prod, qb, kb, OP.mult)
                    # scores: reduce over d (innermost, 32) -> sc [i,j,h]
                    sc = work.tile([P, 4, 4, H], BF16, tag="att_sc")
                    nc.vector.tensor_reduce(
                        out=sc.rearrange("p i j h -> p (i j) h"),
                        in_=prod.rearrange("p i j (h d) -> p (i j) h d", h=H),
                        axis=AX.X, op=OP.add)
                    # fold rstd_i*rstd_j; write [i,h,j] for softmax over j
                    rr2 = work.tile([P, 4, 4], BF16, tag="att_rr2")
                    r1i = rstd1[:, :, None].to_broadcast((P, 4, 4))
                    r1j = rstd1[:, None, :].to_broadcast((P, 4, 4))
                    nc.vector.tensor_tensor(rr2, r1i, r1j, OP.mult)
                    sc2 = work.tile([P, 4, H, 4], BF16, tag="att_sc2")  # [i,h,j]
                    nc.vector.tensor_tensor(
                        sc2.transpose([0, 1, 3, 2]), sc,
                        rr2[:, :, :, None].to_broadcast((P, 4, 4, H)), OP.mult)
                esc = work.tile([P, 4, H, 4], BF16, tag="att_esc")
                nc.scalar.activation(out=esc, in_=sc2, func=AF.Exp, scale=SCALE)
                den = work.tile([P, 4, H], F32, tag="att_den")
                nc.vector.tensor_reduce(out=den, in_=esc, axis=AX.X, op=OP.add)
                rden = work.tile([P, 4, H], F32, tag="att_rden")
                nc.vector.reciprocal_approx_fast(out=rden, in_=den)
                with nc.allow_low_precision(reason="attn bf16"):
                    # pw[i,h,j] = rden[i,h]*rstd1[j]; prob = esc*pw
                    pw = work.tile([P, 4, H, 4], BF16, tag="att_pw")
                    rdb = rden[:, :, :, None].to_broadcast((P, 4, H, 4))
                    rjb = rstd1[:, None, None, :].to_broadcast((P, 4, H, 4))
                    nc.vector.tensor_tensor(pw, rdb, rjb, OP.mult)
                    prob = work.tile([P, 4, H, 4], BF16, tag="att_prob")
                    nc.vector.tensor_tensor(prob, esc, pw, OP.mult)
                    # pv [i,h,d,j] = prob[i,h,j] * vt[h,d,j]; reduce over j
                    pv = attw.tile([P, 4, H, HD, 4], BF16, tag="att_pv")
                    pb = prob[:, :, :, None, :].to_broadcast((P, 4, H, HD, 4))
                    vb = vt[:, None, :, :, :].to_broadcast((P, 4, H, HD, 4))
                    nc.vector.tensor_tensor(pv, pb, vb, OP.mult)
                    pj = attw.tile([P, 4, H, HD, 2], BF16, tag="att_pj")
                    nc.vector.tensor_tensor(pj, pv[:, :, :, :, 0:2],
                                            pv[:, :, :, :, 2:4], OP.add)
                    o = opool.tile([P, 4, D], BF16, tag="att_o")
                    nc.vector.tensor_reduce(
                        out=o.rearrange("p i (h d) -> p i h d", h=H),
                        in_=pj, axis=AX.X, op=OP.add)

                # ---- o @ Wo, residual on GpSimd ----
                oT = lhstp.tile([P, 2, 4, P], BF16, tag="lhst")
                transpose8(o.rearrange("p i (c f) -> p i c f", c=2), oT,
                           "oT", "vector")
                mo = mmpsum.tile([P, 2, 512], F32, tag="mm")
                mov = mo.rearrange("p a (b f) -> p (a b) f", b=2)  # [P,4,256]
                for i in range(4):
                    for c in range(2):
                        nc.tensor.matmul(mov[:, i, :], oT[:, c, i, :],
                                         wo_sb[:, l, c, :],
                                         start=(c == 0), stop=(c == 1))
                with nc.allow_low_precision(reason="bf16 residual"):
                    nc.vector.tensor_tensor(x, x, mov, OP.add)

                # ---- FF ----
                mv2 = _stats4(nc, work, x, f"ln2_{l}")
                rstd2 = _ln_rstd(nc, work, mv2[:, :, 1], 4, eps_t[:, :1],
                                 f"r2_{l}", zero_t[:, :1])
                t2 = xcpool.tile([P, 4, D], BF16, tag="t2")
                with nc.allow_low_precision(reason="ln2 bf16"):
                    for g in range(4):
                        nc.gpsimd.tensor_scalar(
                            out=t2[:, g, :], in0=x[:, g, :],
                            scalar1=mv2[:, g, 0:1], scalar2=rstd2[:, g:g + 1],
                            op0=OP.subtract, op1=OP.mult)
                t2T = lhstp.tile([P, 2, 4, P], BF16, tag="lhst")
                transpose8(t2.rearrange("p i (c f) -> p i c f", c=2), t2T,
                           "t2T", "scalar")
                # W1 weight-stationary: z1T [P(ff in chunk fc), fc, (i r)]
                mz = mmpsum.tile([P, 2, 512], F32, tag="mm")
                for fc in range(2):
                    for c in range(2):
                        nc.tensor.matmul(
                            mz[:, fc, :], w1_sb[:, l, c, fc, :],
                            t2T[:, c, :, :].rearrange("p i f -> p (i f)"),
                            start=(c == 0), stop=(c == 1))
                # gelu via odd erf-poly (DVE, bf16): gl = z*(1+erf(z/sqrt2))
                # with the 1/2 folded into W2 on host.
                z1c = glpool.tile([P, 2, 512], BF16, tag="z1c")
                nc.scalar.copy(out=z1c, in_=mz)
                gl = glpool.tile([P, 2, 512], BF16, tag="gl")
                with nc.allow_low_precision(reason="gelu bf16"):
                    # Horner: erf = z*(a1 + z2*(a3 + z2*(a5 + z2*a7)))
                    z2 = glpool.tile([P, 2, 512], BF16, tag="z2")
                    nc.gpsimd.tensor_tensor(z2, z1c, z1c, OP.mult)
                    h1 = glpool.tile([P, 2, 512], BF16, tag="h1")
                    nc.gpsimd.tensor_scalar(out=h1, in0=z2, scalar1=ERF_A7,
                                            scalar2=ERF_A5, op0=OP.mult,
                                            op1=OP.add)
                    h2 = glpool.tile([P, 2, 512], BF16, tag="h2")
                    nc.gpsimd.tensor_tensor(h2, h1, z2, OP.mult)
                    h3 = glpool.tile([P, 2, 512], BF16, tag="h3")
                    nc.vector.scalar_tensor_tensor(
                        out=h3, in0=h2, scalar=ERF_A3, in1=z2,
                        op0=OP.add, op1=OP.mult)
                    w_ = glpool.tile([P, 2, 512], BF16, tag="w_")
                    nc.vector.scalar_tensor_tensor(
                        out=w_, in0=h3, scalar=ERF_A1, in1=z1c,
                        op0=OP.add, op1=OP.mult)
                    nc.vector.scalar_tensor_tensor(
                        out=gl, in0=w_, scalar=1.0, in1=z1c,
                        op0=OP.add, op1=OP.mult)
                glv = gl.rearrange("p c (i f) -> p c i f", i=4)
                mw = mmpsum.tile([P, 2, 512], F32, tag="mm")
                mwv = mw.rearrange("p a (b f) -> p (a b) f", b=2)  # [P,4,256]
                for i in range(4):
                    for fc in range(2):
                        nc.tensor.matmul(mwv[:, i, :], glv[:, fc, i, :],
                                         w2_sb[:, l, fc, :],
                                         start=(fc == 0), stop=(fc == 1))
                with nc.allow_low_precision(reason="bf16 residual"):
                    nc.vector.tensor_tensor(x, x, mwv, OP.add)

            # ---- tail: final_ln per token, mean/4, out_ln ----
            mvf = _stats4(nc, work, x, "fin")
            # fold the 1/4 of the token mean into rstd: exp bias ln(1/4)
            rstdf = _ln_rstd(nc, work, mvf[:, :, 1], 4, eps_t[:, :1], "rf",
                             lnq_t[:, :1])
            xt = work.tile([P, 4, D], BF16, tag="tail_xt")
            with nc.allow_low_precision(reason="tail bf16"):
                for g in range(4):
                    nc.vector.tensor_scalar(
                        out=xt[:, g, :], in0=x[:, g, :],
                        scalar1=mvf[:, g, 0:1], scalar2=rstdf[:, g:g + 1],
                        op0=OP.subtract, op1=OP.mult)
                u1 = work.tile([P, 2, D], BF16, tag="tail_u1")
                nc.vector.tensor_tensor(u1, xt[:, 0:2, :], xt[:, 2:4, :], OP.add)
                u = work.tile([P, D], BF16, tag="tail_u")
                nc.vector.tensor_tensor(u, u1[:, 0, :], u1[:, 1, :], OP.add)
            st6f = work.tile([P, 6], F32, tag="out_st")
            nc.vector.bn_stats(out=st6f, in_=u)
            mvo = work.tile([P, 2], F32, tag="out_mv")
            nc.vector.bn_aggr(out=mvo, in_=st6f)
            rstdo = _ln_rstd(nc, work, mvo[:, 1:2], 1, eps_t[:, :1], "ro", zero_t[:, :1])
            res = opool.tile([P, D], F32, tag="res")
            nc.vector.tensor_scalar(out=res, in0=u, scalar1=mvo[:, 0:1],
                                    scalar2=rstdo[:, 0:1], op0=OP.subtract,
                                    op1=OP.mult)
            nc.sync.dma_start(out=out[row:row + P, :], in_=res)

    return nc


def _fold_host(inputs):
    f = lambda k: np.asarray(inputs[k], dtype=np.float32)
    # -- assert the structural zeros/ones this kernel folds away --
    assert not np.any(f("bqkv")) and not np.any(f("bo")), "nonzero qkv/o bias"
    assert not np.any(f("b1")) and not np.any(f("b2")), "nonzero ff bias"
    assert not np.any(f("ln1_b")) and not np.any(f("ln2_b")), "nonzero ln bias"
    assert not np.any(f("sym_b")), "nonzero sym_b"
    assert np.allclose(f("sym_ln_g"), 1.0), "sym_ln_g != 1"
    assert np.allclose(f("final_ln_g"), 1.0) and not np.any(f("final_ln_b"))
    assert np.allclose(f("out_ln_g"), 1.0) and not np.any(f("out_ln_b"))

    g1, g2 = f("ln1_g"), f("ln2_g")
    wqkv = g1[:, :, None] * f("Wqkv")          # [L, D, 3D]
    w1 = g2[:, :, None] * f("W1")              # [L, D, FF]
    w2 = 0.5 * f("W2")                         # gelu's 1/2 folded here
    wo = f("Wo")

    tte = f("token_type_emb")
    Bsz = B
    X = np.empty((Bsz, 4, D), dtype=np.float32)
    X[:, 0] = f("global_emb") + tte[0]
    X[:, 1] = f("pert_emb") + tte[1]
    X[:, 2] = 0.0
    X[:, 3] = f("ppi_feat") + tte[3]

    sfp = np.zeros((Bsz, P), dtype=np.float32)
    sfp[:, :SYM] = f("sym_feat")

    symw = np.zeros((P, D), dtype=np.float32)
    symw[:SYM] = f("sym_W")

    vecb = (f("sym_ln_b") + tte[2]).reshape(1, D)

    ch = lambda w: np.ascontiguousarray(w.reshape(L, 2, P, -1))
    w1c = np.ascontiguousarray(
        w1.reshape(L, 2, P, 2, P).transpose(0, 1, 3, 2, 4))  # [L,dc,fc,128,128]

    bf = lambda a: np.ascontiguousarray(a.astype(BF))
    return dict(
        xin=bf(X), sfp=bf(sfp), symw=bf(symw), vecb=bf(vecb),
        wqkv=bf(ch(wqkv)), wo=bf(ch(wo)), w1=bf(w1c), w2=bf(ch(w2)),
    )


_CACHE = {}


def _get_built():
    key = "k2"
    if key not in _CACHE:
        from concourse import bacc
        nc = bacc.Bacc("TRN2", target_bir_lowering=False, debug=False,
                       num_devices=NCORES)
        build_kernel(nc)
        nc.compile()
        _CACHE[key] = nc
    return _CACHE[key]


def kernel(**inputs):
    fold = _fold_host(inputs)
    nc = _get_built()

    shared = {k: fold[k] for k in
              ("symw", "vecb", "wqkv", "wo", "w1", "w2")}
    in_maps = []
    for c in range(NCORES):
        sl = slice(c * BC, (c + 1) * BC)
        m = dict(shared)
        m["xin"] = np.ascontiguousarray(fold["xin"][sl])
        m["sfp"] = np.ascontiguousarray(fold["sfp"][sl])
        in_maps.append(m)

    res = run_bass_kernel_spmd(nc, in_maps, core_ids=list(range(NCORES)))
    global LAST_RESULT
    LAST_RESULT = res
    outs = [res.results[c]["out"] for c in range(NCORES)]
    return np.concatenate(outs, axis=0)


LAST_RESULT = None


if __name__ == "__main__":
    print("smoke build only")
    _get_built()
    print("built ok")


# revision 10
# speedup vs baseline: 1.8474x; 1.8474x over previous
"""Trainium2 Bass kernel for nn_CrossAttentionFusion (dense_transformer).

Pure data parallel over 8 NeuronCores (batch 32768 -> 4096/core), 32 tiles of
128 rows each.  Row-major residual stream in bf16; attention on the Vector
engine with packed-bf16 access patterns (2x/4x DVE modes); matmuls on PE in
bf16 (activation-stationary for QKV/Wo/W2, weight-stationary for W1 so the
gelu output is directly the W2 lhsT).  LN1's per-row rstd is folded into the
softmax (rstd_i*rstd_j on scores, rstd_j into prob) so LN1's apply never
materializes.  All Scalar-engine activations draw from one table set
(ln+exp): rsqrt = exp(-0.5*ln(v+eps)); gelu is an erf-polynomial on DVE.
Residual adds and small copies ride the otherwise-idle GpSimd engine.
"""

import contextlib
import ctypes
import math
import os
import sys
import types
from contextlib import ExitStack

import numpy as np
import ml_dtypes

import concourse.bass as bass
import concourse.tile as tile
from concourse import mybir
from concourse.bass_utils import run_bass_kernel_spmd
from concourse.masks import make_identity


def _install_ntff_hook_shim():
    """Provide antenv.axon_hooks if the image lacks it, so trace=True works."""
    try:
        import antenv.axon_hooks  # noqa: F401
        return
    except ImportError:
        pass
    so_path = "/opt/axon/libaxon_pjrt.so"
    hook = None
    if os.path.exists(so_path):
        try:
            lib = ctypes.CDLL(so_path)
            if hasattr(lib, "axon_start_nrt_profile"):
                lib.axon_start_nrt_profile.argtypes = [
                    ctypes.POINTER(ctypes.c_int64), ctypes.c_size_t]
                lib.axon_start_nrt_profile.restype = ctypes.c_int64
                lib.axon_stop_nrt_profile.argtypes = [ctypes.c_char_p]
                lib.axon_stop_nrt_profile.restype = ctypes.c_int64

                @contextlib.contextmanager
                def _hook(output_dir, device_ids):
                    import jax
                    jax.devices()
                    if device_ids:
                        ids = (ctypes.c_int64 * len(device_ids))(*device_ids)
                        rc = lib.axon_start_nrt_profile(ids, len(device_ids))
                    else:
                        rc = lib.axon_start_nrt_profile(None, 0)
                    if rc != 0:
                        raise RuntimeError(f"axon_start_nrt_profile rc={rc}")
                    try:
                        yield
                    finally:
                        n = lib.axon_stop_nrt_profile(str(output_dir).encode())
                        print(f"ntff profile: {n} file(s) -> {output_dir}",
                              file=sys.stderr)

                hook = _hook
        except OSError:
            pass

    mod = types.ModuleType("antenv.axon_hooks")
    mod.get_axon_ntff_profile_hook = lambda: hook
    mod.set_axon_ntff_profile_hook = lambda h: None
    sys.modules["antenv.axon_hooks"] = mod


_install_ntff_hook_shim()

# Problem shapes (hardcoded per contract).
D, H, HD, FF, L, SYM, B = 256, 8, 32, 256, 3, 64, 32768
NCORES = 8
BC = B // NCORES          # 4096 rows per core
P = 128                   # SBUF partitions
NT = BC // P              # 32 tiles per core
F32 = mybir.dt.float32
BF16 = mybir.dt.bfloat16
AF = mybir.ActivationFunctionType
OP = mybir.AluOpType
AX = mybir.AxisListType
EPS = 1e-5
SCALE = 1.0 / math.sqrt(HD)
GA = 1.702  # unused (erf-poly gelu); kept for reference

# odd-polynomial fit of erf(z/sqrt(2)) on |z|<=2.6 (max err 3e-3; the gelu
# input z1 has std ~0.32 so 6-sigma is ~1.9)
ERF_A1 = 0.79397813
ERF_A3 = -0.12376735
ERF_A5 = 0.013831441
ERF_A7 = -6.7821721e-4

BF = ml_dtypes.bfloat16


def _ln_rstd(nc, work, mv_var_ap, n, eps_ap, tag, bias_ap=0.0):
    """rstd = exp(-0.5*ln(var+eps) + bias) on Scalar (single-table)."""
    lnv = work.tile([P, n], F32, tag=tag + "_lnv")
    nc.scalar.activation(out=lnv, in_=mv_var_ap, func=AF.Ln,
                         bias=eps_ap, scale=1.0)
    rstd = work.tile([P, n], F32, tag=tag + "_rstd")
    nc.scalar.activation(out=rstd, in_=lnv, func=AF.Exp, scale=-0.5,
                         bias=bias_ap)
    return rstd


def _stats4(nc, work, x, tag):
    """bn stats for 4 groups of 256. Returns mv [P,4,2] (mean,var)."""
    st = work.tile([P, 4, 6], F32, tag=tag + "_st")
    for g in range(4):
        nc.vector.bn_stats(out=st[:, g, :], in_=x[:, g, :])
    mv = work.tile([P, 4, 2], F32, tag=tag + "_mv")
    for g in range(4):
        nc.vector.bn_aggr(out=mv[:, g, :], in_=st[:, g, :])
    return mv


def build_kernel(nc):
    # Per-core data inputs (host pre-adds token-type emb, casts to bf16,
    # zero-pads sym_feat 64->128 and x slot 2).
    xin = nc.dram_tensor("xin", [BC, 4, D], BF16, kind="ExternalInput").ap()
    sfp = nc.dram_tensor("sfp", [BC, P], BF16, kind="ExternalInput").ap()
    # Replicated weights, bf16, pre-chunked for 128-partition contractions.
    symw = nc.dram_tensor("symw", [P, D], BF16, kind="ExternalInput").ap()
    wqkv = nc.dram_tensor("wqkv", [L, 2, P, 3 * D], BF16, kind="ExternalInput").ap()
    wo = nc.dram_tensor("wo", [L, 2, P, D], BF16, kind="ExternalInput").ap()
    w1 = nc.dram_tensor("w1", [L, 2, 2, P, P], BF16, kind="ExternalInput").ap()
    w2 = nc.dram_tensor("w2", [L, 2, P, D], BF16, kind="ExternalInput").ap()
    vecb = nc.dram_tensor("vecb", [1, D], BF16, kind="ExternalInput").ap()  # symbt
    out = nc.dram_tensor("out", [BC, D], F32, kind="ExternalOutput").ap()

    with ExitStack() as ctx:
        tc = ctx.enter_context(tile.TileContext(nc))
        singles = ctx.enter_context(tc.tile_pool(name="singles", bufs=1))
        work = ctx.enter_context(tc.tile_pool(name="work", bufs=4))
        xpool = ctx.enter_context(tc.tile_pool(name="xpool", bufs=2))
        xcpool = ctx.enter_context(tc.tile_pool(name="xcpool", bufs=2))
        lhstp = ctx.enter_context(tc.tile_pool(name="lhst", bufs=2))
        qkpool = ctx.enter_context(tc.tile_pool(name="qkpool", bufs=2))
        vtpool = ctx.enter_context(tc.tile_pool(name="vtpool", bufs=2))
        attw = ctx.enter_context(tc.tile_pool(name="attw", bufs=2))
        opool = ctx.enter_context(tc.tile_pool(name="opool", bufs=2))
        glpool = ctx.enter_context(tc.tile_pool(name="glpool", bufs=2))
        tpsum = ctx.enter_context(tc.tile_pool(name="tpsum", bufs=2, space="PSUM"))
        mmpsum = ctx.enter_context(tc.tile_pool(name="mmpsum", bufs=3, space="PSUM"))

        # ---- constants / resident weights ----
        identb = singles.tile([P, P], BF16)
        make_identity(nc, identb)
        eps_t = singles.tile([P, 1], F32)
        nc.vector.memset(eps_t, EPS)
        zero_t = singles.tile([P, 1], F32)
        nc.vector.memset(zero_t, 0.0)
        lnq_t = singles.tile([P, 1], F32)
        nc.vector.memset(lnq_t, math.log(0.25))
        symw_sb = singles.tile([P, D], BF16)
        nc.gpsimd.dma_start(out=symw_sb, in_=symw)
        wqkv_sb = singles.tile([P, L, 2, 3 * D], BF16)
        nc.gpsimd.dma_start(out=wqkv_sb, in_=wqkv.transpose([2, 0, 1, 3]))
        wo_sb = singles.tile([P, L, 2, D], BF16)
        nc.gpsimd.dma_start(out=wo_sb, in_=wo.transpose([2, 0, 1, 3]))
        w1_sb = singles.tile([P, L, 2, 2, P], BF16)
        nc.gpsimd.dma_start(out=w1_sb, in_=w1.transpose([3, 0, 1, 2, 4]))
        w2_sb = singles.tile([P, L, 2, D], BF16)
        nc.gpsimd.dma_start(out=w2_sb, in_=w2.transpose([2, 0, 1, 3]))
        symbt_sb = singles.tile([P, 1, D], BF16)
        nc.sync.dma_start(out=symbt_sb, in_=vecb.partition_broadcast(P))

        def transpose8(src, dst, tag, copy_engine):
            """src: [P, 4(i), 2(c), 128] bf16 view; dst: [P, 2(c), 4(i), 128]
            SBUF tile with dst[:, c, i, :] = src[:, i, c, :].T"""
            for c in range(2):
                pt = tpsum.tile([P, 4, P], BF16, tag="tp")
                for i in range(4):
                    nc.tensor.transpose(pt[:, i, :], src[:, i, c, :], identb)
                ce = getattr(nc, copy_engine)
                if copy_engine == "scalar":
                    ce.copy(out=dst[:, c], in_=pt)
                else:
                    with nc.allow_low_precision(reason="bf16 lhsT copy"):
                        ce.tensor_copy(out=dst[:, c], in_=pt)

        for it in range(NT):
            row = it * P
            # ---- build x [P,4,D] bf16 (slot 2 incoming zeros; sym fills it) --
            x = xpool.tile([P, 4, D], BF16, tag="x")
            nc.sync.dma_start(out=x, in_=xin[row:row + P])
            sft = work.tile([P, P], BF16, tag="sft")
            nc.sync.dma_start(out=sft, in_=sfp[row:row + P])

            # sym branch: x2 = LN(sf @ symW) + symbt   (sym_ln_g==1 asserted host)
            pt = tpsum.tile([P, 4, P], BF16, tag="tp")
            nc.tensor.transpose(pt[:, 0, :], sft, identb)
            sfT = work.tile([P, P], BF16, tag="sfT")
            nc.scalar.copy(out=sfT, in_=pt[:, 0, :])
            mm = mmpsum.tile([P, 2, 512], F32, tag="mm")
            zsym = mm[:, 0, 0:D]
            nc.tensor.matmul(zsym, sfT, symw_sb, start=True, stop=True)
            st6 = work.tile([P, 6], F32, tag="sym_st")
            nc.vector.bn_stats(out=st6, in_=zsym)
            mvs = work.tile([P, 2], F32, tag="sym_mv")
            nc.vector.bn_aggr(out=mvs, in_=st6)
            rstd_s = _ln_rstd(nc, work, mvs[:, 1:2], 1, eps_t[:, :1], "sym", zero_t[:, :1])
            zn = work.tile([P, D], BF16, tag="sym_zn")
            nc.vector.tensor_scalar(out=zn, in0=zsym, scalar1=mvs[:, 0:1],
                                    scalar2=rstd_s[:, 0:1], op0=OP.subtract,
                                    op1=OP.mult)
            with nc.allow_low_precision(reason="bf16 residual stream"):
                nc.vector.tensor_tensor(x[:, 2, :], zn, symbt_sb[:, 0, :], OP.add)

            # ---- transformer layers ----
            for l in range(L):
                # LN1 stats; apply is folded into attention scalars.
                mv1 = _stats4(nc, work, x, f"ln1_{l}")
                rstd1 = _ln_rstd(nc, work, mv1[:, :, 1], 4, eps_t[:, :1], f"r1_{l}", zero_t[:, :1])
                xc = xcpool.tile([P, 4, D], BF16, tag="xc")
                with nc.allow_low_precision(reason="centered acts bf16"):
                    for g in range(4):
                        nc.vector.tensor_scalar(
                            out=xc[:, g, :], in0=x[:, g, :],
                            scalar1=mv1[:, g, 0:1], scalar2=None,
                            op0=OP.subtract)
                # xcT [P, 2(c), 4(i), 128]
                xcT = lhstp.tile([P, 2, 4, P], BF16, tag="lhst")
                transpose8(xc.rearrange("p i (c f) -> p i c f", c=2), xcT,
                           "xcT", "scalar")

                # qkv per token i: q|k -> qk sbuf, v -> vt[h,d,j=i]
                qk = qkpool.tile([P, 4, 512], BF16, tag="qk")
                vt = vtpool.tile([P, H, HD, 4], BF16, tag="vt")
                for i in range(4):
                    mmi = mmpsum.tile([P, 2, 512], F32, tag="mm")
                    for c in range(2):
                        nc.tensor.matmul(mmi[:, 0, :], xcT[:, c, i, :],
                                         wqkv_sb[:, l, c, 0:512],
                                         start=(c == 0), stop=(c == 1))
                    for c in range(2):
                        nc.tensor.matmul(mmi[:, 1, 0:D], xcT[:, c, i, :],
                                         wqkv_sb[:, l, c, 512:768],
                                         start=(c == 0), stop=(c == 1))
                    nc.scalar.copy(out=qk[:, i, :], in_=mmi[:, 0, :])
                    nc.scalar.copy(
                        out=vt[:, :, :, i],
                        in_=mmi[:, 1, 0:D].rearrange("p (h d) -> p h d", h=H))

                # ---- attention (row-major, packed bf16) ----
                q = qk[:, :, 0:D]       # [P, i, (h d)]
                k = qk[:, :, D:2 * D]   # [P, j, (h d)]
                prod = attw.tile([P, 4, 4, D], BF16, tag="att_prod")  # [i,j,hd]
                qb = q[:, :, None, :].to_broadcast((P, 4, 4, D))
                kb = k[:, None, :, :].to_broadcast((P, 4, 4, D))
                with nc.allow_low_precision(reason="attn bf16"):
                    nc.vector.tensor_tensor(prod, qb, kb, OP.mult)
                    # scores: reduce over d (innermost, 32) -> sc [i,j,h]
                    pr = prod.rearrange("p i j (h d) -> p (i j) h d", h=H)
                    tr16 = attw.tile([P, 16, H, 16], BF16, tag="att_tr16")
                    nc.vector.tensor_tensor(tr16, pr[:, :, :, 0:16],
                                            pr[:, :, :, 16:32], OP.add)
                    tr4 = work.tile([P, 16, H, 4], BF16, tag="att_tr4")
                    t8 = tr16[:, :, :, 0:8]
                    nc.vector.tensor_tensor(t8, tr16[:, :, :, 0:8],
                                            tr16[:, :, :, 8:16], OP.add)
                    nc.vector.tensor_tensor(tr4, t8[:, :, :, 0:4],
                                            t8[:, :, :, 4:8], OP.add)
                    tr2 = work.tile([P, 16, H, 2], BF16, tag="att_tr2")
                    nc.vector.tensor_tensor(tr2, tr4[:, :, :, 0:2],
                                            tr4[:, :, :, 2:4], OP.add)
                    sc = work.tile([P, 4, 4, H], BF16, tag="att_sc")
                    nc.vector.tensor_reduce(
                        out=sc.rearrange("p i j h -> p (i j) h"),
                        in_=tr2, axis=AX.X, op=OP.add)
                    # fold rstd_i*rstd_j; write [i,h,j] for softmax over j
                    rr2 = work.tile([P, 4, 4], BF16, tag="att_rr2")
                    r1i = rstd1[:, :, None].to_broadcast((P, 4, 4))
                    r1j = rstd1[:, None, :].to_broadcast((P, 4, 4))
                    nc.vector.tensor_tensor(rr2, r1i, r1j, OP.mult)
                    sc2 = work.tile([P, 4, H, 4], BF16, tag="att_sc2")  # [i,h,j]
                    nc.vector.tensor_tensor(
                        sc2.transpose([0, 1, 3, 2]), sc,
                        rr2[:, :, :, None].to_broadcast((P, 4, 4, H)), OP.mult)
                esc = work.tile([P, 4, H, 4], BF16, tag="att_esc")
                nc.scalar.activation(out=esc, in_=sc2, func=AF.Exp, scale=SCALE)
                den = work.tile([P, 4, H], F32, tag="att_den")
                nc.vector.tensor_reduce(out=den, in_=esc, axis=AX.X, op=OP.add)
                rden = work.tile([P, 4, H], F32, tag="att_rden")
                nc.vector.reciprocal_approx_fast(out=rden, in_=den)
                with nc.allow_low_precision(reason="attn bf16"):
                    # pw[i,h,j] = rden[i,h]*rstd1[j]; prob = esc*pw
                    pw = work.tile([P, 4, H, 4], BF16, tag="att_pw")
                    rdb = rden[:, :, :, None].to_broadcast((P, 4, H, 4))
                    rjb = rstd1[:, None, None, :].to_broadcast((P, 4, H, 4))
                    nc.vector.tensor_tensor(pw, rdb, rjb, OP.mult)
                    prob = work.tile([P, 4, H, 4], BF16, tag="att_prob")
                    nc.vector.tensor_tensor(prob, esc, pw, OP.mult)
                    # pv [i,h,d,j] = prob[i,h,j] * vt[h,d,j]; reduce over j
                    pv = attw.tile([P, 4, H, HD, 4], BF16, tag="att_pv")
                    pb = prob[:, :, :, None, :].to_broadcast((P, 4, H, HD, 4))
                    vb = vt[:, None, :, :, :].to_broadcast((P, 4, H, HD, 4))
                    nc.vector.tensor_tensor(pv, pb, vb, OP.mult)
                    pj = attw.tile([P, 4, H, HD, 2], BF16, tag="att_pj")
                    nc.vector.tensor_tensor(pj, pv[:, :, :, :, 0:2],
                                            pv[:, :, :, :, 2:4], OP.add)
                    o = opool.tile([P, 4, D], BF16, tag="att_o")
                    nc.vector.tensor_tensor(
                        o.rearrange("p i (h d) -> p i h d", h=H),
                        pj[:, :, :, :, 0], pj[:, :, :, :, 1], OP.add)

                # ---- o @ Wo, residual on GpSimd ----
                oT = lhstp.tile([P, 2, 4, P], BF16, tag="lhst")
                transpose8(o.rearrange("p i (c f) -> p i c f", c=2), oT,
                           "oT", "vector")
                mo = mmpsum.tile([P, 2, 512], F32, tag="mm")
                mov = mo.rearrange("p a (b f) -> p (a b) f", b=2)  # [P,4,256]
                for i in range(4):
                    for c in range(2):
                        nc.tensor.matmul(mov[:, i, :], oT[:, c, i, :],
                                         wo_sb[:, l, c, :],
                                         start=(c == 0), stop=(c == 1))
                with nc.allow_low_precision(reason="bf16 residual"):
                    nc.vector.tensor_tensor(x, x, mov, OP.add)

                # ---- FF ----
                mv2 = _stats4(nc, work, x, f"ln2_{l}")
                rstd2 = _ln_rstd(nc, work, mv2[:, :, 1], 4, eps_t[:, :1],
                                 f"r2_{l}", zero_t[:, :1])
                t2 = xcpool.tile([P, 4, D], BF16, tag="t2")
                with nc.allow_low_precision(reason="ln2 bf16"):
                    for g in range(4):
                        nc.vector.tensor_scalar(
                            out=t2[:, g, :], in0=x[:, g, :],
                            scalar1=mv2[:, g, 0:1], scalar2=rstd2[:, g:g + 1],
                            op0=OP.subtract, op1=OP.mult)
                t2T = lhstp.tile([P, 2, 4, P], BF16, tag="lhst")
                transpose8(t2.rearrange("p i (c f) -> p i c f", c=2), t2T,
                           "t2T", "scalar")
                # W1 weight-stationary: z1T [P(ff in chunk fc), fc, (i r)]
                mz = mmpsum.tile([P, 2, 512], F32, tag="mm")
                for fc in range(2):
                    for c in range(2):
                        nc.tensor.matmul(
                            mz[:, fc, :], w1_sb[:, l, c, fc, :],
                            t2T[:, c, :, :].rearrange("p i f -> p (i f)"),
                            start=(c == 0), stop=(c == 1))
                # gelu ~= z*sigmoid(1.702 z): e = exp(-1.702 z) on Scalar
                # (same table as softmax), sigma = recip(1+e) on DVE,
                # gl = z * sigma with z read from PSUM.  1/2 NOT folded:
                # sigmoid form needs no 1/2.
                e_t = glpool.tile([P, 2, 512], BF16, tag="e_t")
                nc.scalar.activation(out=e_t, in_=mz, func=AF.Exp, scale=-GA)
                d_t = glpool.tile([P, 2, 512], F32, tag="d_t")
                nc.vector.tensor_scalar(out=d_t, in0=e_t, scalar1=1.0,
                                        scalar2=None, op0=OP.add)
                r_t = glpool.tile([P, 2, 512], F32, tag="r_t")
                nc.vector.reciprocal_approx_fast(out=r_t, in_=d_t)
                gl = glpool.tile([P, 2, 512], BF16, tag="gl")
                with nc.allow_low_precision(reason="gelu bf16"):
                    nc.vector.tensor_tensor(gl, mz, r_t, OP.mult)
                glv = gl.rearrange("p c (i f) -> p c i f", i=4)
                mw = mmpsum.tile([P, 2, 512], F32, tag="mm")
                mwv = mw.rearrange("p a (b f) -> p (a b) f", b=2)  # [P,4,256]
                for i in range(4):
                    for fc in range(2):
                        nc.tensor.matmul(mwv[:, i, :], glv[:, fc, i, :],
                                         w2_sb[:, l, fc, :],
                                         start=(fc == 0), stop=(fc == 1))
                with nc.allow_low_precision(reason="bf16 residual"):
                    nc.vector.tensor_tensor(x, x, mwv, OP.add)

            # ---- tail: final_ln per token, mean/4, out_ln ----
            mvf = _stats4(nc, work, x, "fin")
            # fold the 1/4 of the token mean into rstd: exp bias ln(1/4)
            rstdf = _ln_rstd(nc, work, mvf[:, :, 1], 4, eps_t[:, :1], "rf",
                             lnq_t[:, :1])
            xt = work.tile([P, 4, D], BF16, tag="tail_xt")
            with nc.allow_low_precision(reason="tail bf16"):
                for g in range(4):
                    nc.vector.tensor_scalar(
                        out=xt[:, g, :], in0=x[:, g, :],
                        scalar1=mvf[:, g, 0:1], scalar2=rstdf[:, g:g + 1],
                        op0=OP.subtract, op1=OP.mult)
                u1 = work.tile([P, 2, D], BF16, tag="tail_u1")
                nc.vector.tensor_tensor(u1, xt[:, 0:2, :], xt[:, 2:4, :], OP.add)
                u = work.tile([P, D], BF16, tag="tail_u")
                nc.vector.tensor_tensor(u, u1[:, 0, :], u1[:, 1, :], OP.add)
            st6f = work.tile([P, 6], F32, tag="out_st")
            nc.vector.bn_stats(out=st6f, in_=u)
            mvo = work.tile([P, 2], F32, tag="out_mv")
            nc.vector.bn_aggr(out=mvo, in_=st6f)
            rstdo = _ln_rstd(nc, work, mvo[:, 1:2], 1, eps_t[:, :1], "ro", zero_t[:, :1])
            res = opool.tile([P, D], F32, tag="res")
            nc.vector.tensor_scalar(out=res, in0=u, scalar1=mvo[:, 0:1],
                                    scalar2=rstdo[:, 0:1], op0=OP.subtract,
                                    op1=OP.mult)
            nc.sync.dma_start(out=out[row:row + P, :], in_=res)

    return nc


def _fold_host(inputs):
    f = lambda k: np.asarray(inputs[k], dtype=np.float32)
    # -- assert the structural zeros/ones this kernel folds away --
    assert not np.any(f("bqkv")) and not np.any(f("bo")), "nonzero qkv/o bias"
    assert not np.any(f("b1")) and not np.any(f("b2")), "nonzero ff bias"
    assert not np.any(f("ln1_b")) and not np.any(f("ln2_b")), "nonzero ln bias"
    assert not np.any(f("sym_b")), "nonzero sym_b"
    assert np.allclose(f("sym_ln_g"), 1.0), "sym_ln_g != 1"
    assert np.allclose(f("final_ln_g"), 1.0) and not np.any(f("final_ln_b"))
    assert np.allclose(f("out_ln_g"), 1.0) and not np.any(f("out_ln_b"))

    g1, g2 = f("ln1_g"), f("ln2_g")
    wqkv = g1[:, :, None] * f("Wqkv")          # [L, D, 3D]
    w1 = g2[:, :, None] * f("W1")              # [L, D, FF]
    w2 = f("W2")
    wo = f("Wo")

    tte = f("token_type_emb")
    Bsz = B
    X = np.empty((Bsz, 4, D), dtype=np.float32)
    X[:, 0] = f("global_emb") + tte[0]
    X[:, 1] = f("pert_emb") + tte[1]
    X[:, 2] = 0.0
    X[:, 3] = f("ppi_feat") + tte[3]

    sfp = np.zeros((Bsz, P), dtype=np.float32)
    sfp[:, :SYM] = f("sym_feat")

    symw = np.zeros((P, D), dtype=np.float32)
    symw[:SYM] = f("sym_W")

    vecb = (f("sym_ln_b") + tte[2]).reshape(1, D)

    ch = lambda w: np.ascontiguousarray(w.reshape(L, 2, P, -1))
    w1c = np.ascontiguousarray(
        w1.reshape(L, 2, P, 2, P).transpose(0, 1, 3, 2, 4))  # [L,dc,fc,128,128]

    bf = lambda a: np.ascontiguousarray(a.astype(BF))
    return dict(
        xin=bf(X), sfp=bf(sfp), symw=bf(symw), vecb=bf(vecb),
        wqkv=bf(ch(wqkv)), wo=bf(ch(wo)), w1=bf(w1c), w2=bf(ch(w2)),
    )


_CACHE = {}


def _patch_act_table_choice():
    """Prefer natural_log_exp_and_others for ln/exp/identity/copy so the
    Ln<->Exp alternation never reloads activation tables.  Only the set
    SELECTION heuristic changes: entries keep their positions, so the
    act_func_set_id written into BIR stays a truthful index."""
    import concourse.bacc as bacc_mod
    real = bacc_mod.get_activation_tables
    target = "natural_log_exp_and_others"

    def patched(arch):
        tabs = real(arch)
        items = list(tabs.items())
        names = [n for n, _ in items]
        if target not in names:
            return tabs
        ti = names.index(target)
        tfuncs = items[ti][1]
        out = {}
        for idx, (n, fs) in enumerate(items):
            out[n] = (fs - tfuncs) if idx < ti else fs
        return out

    bacc_mod.get_activation_tables = patched


def _get_built():
    key = "k3"
    if key not in _CACHE:
        from concourse import bacc
        _patch_act_table_choice()
        nc = bacc.Bacc("TRN2", target_bir_lowering=False, debug=False,
                       num_devices=NCORES)
        build_kernel(nc)
        nc.compile()
        _CACHE[key] = nc
    return _CACHE[key]


def kernel(**inputs):
    fold = _fold_host(inputs)
    nc = _get_built()

    shared = {k: fold[k] for k in
              ("symw", "vecb", "wqkv", "wo", "w1", "w2")}
    in_maps = []
    for c in range(NCORES):
        sl = slice(c * BC, (c + 1) * BC)
        m = dict(shared)
        m["xin"] = np.ascontiguousarray(fold["xin"][sl])
        m["sfp"] = np.ascontiguousarray(fold["sfp"][sl])
        in_maps.append(m)

    res = run_bass_kernel_spmd(nc, in_maps, core_ids=list(range(NCORES)))
    global LAST_RESULT
    LAST_RESULT = res
    outs = [res.results[c]["out"] for c in range(NCORES)]
    return np.concatenate(outs, axis=0)


LAST_RESULT = None


if __name__ == "__main__":
    print("smoke build only")
    _get_built()
    print("built ok")
